# revision 44
# baseline (speedup 1.0000x reference)
"""Single-head attention layer on 8 TRN2 NeuronCores.

Data-parallel over batch: B=8 batch elements, one per core. Each core
computes, for its x [S=2048, E=1024] slice:
    Q = x@Wq+bq; K = x@Wk+bk; V = x@Wv+bv        (KQ = VDIM = 128)
    O = softmax(Q K^T / sqrt(128)) V @ Wo + bo
All matmuls run in bf16 with fp32 PSUM accumulation (measured L2 rel
err ~1e-3 vs the f32 reference). Softmax skips the max-subtraction
(scores are in [-2.5, 2.5] for this input distribution) so the row sum
can be computed with a ones-vector matmul and normalization folds into
the H^T PSUM->SBUF copy.

Perf notes (trace-driven; 137us -> ~110us; measured exec has +-2.5us
run-to-run noise, so treat single samples with care):
- Measured PE cadence model: MM duration = ~165ns + cols*0.42ns, but
  back-to-back MMs PIPELINE: sustained spacing 216ns (512-col) /
  56ns (128-col) at full clock. The PE runs at HALF clock until ~3us
  of CONTINUOUS execution (gaps reset the ramp; full speed lands
  ~24.5us because the early x arrivals trickle). Consecutive scalar
  exps pipeline at ~570ns despite 686ns durations.
- ALL HBM reads ride ONE gpsimd cast-DMA stream in PE-consumption
  order (x tiles and weights interleaved; each W one DMA op). One
  queue saturates the ~350 GB/s per-core HBM port. DMA-op completion
  semaphores fire in ISSUE order with multi-us lag on the first few
  ops -- reordering early loads to "arrive later but denser"
  backfires. A tiny warm-up read absorbs first-transfer latency.
- The first matmul in PE program order inherits a spurious wait on
  the gpsimd DMA-queue op semaphore (fires ~8.6-16us); anything
  data-independent (bo broadcast) must NOT sit at the PE head.
- x^T is built per-stile on the PE (8 transposes into ONE
  [128,1024]bf16 bank = one 2KB PSUM bank), and the K/Q projections
  run PER-STILE (8x 128-col MMs each, half-interleaved K/Q so the
  second xT half-copy is never waited on). Stiles 0-3 project
  0-behind their transpose to fill the DMA-paced head gaps; stiles
  4+ project 1-behind so the DVE xT copies are fully hidden.
- V is computed directly in NATURAL [s,v] layout from the xT slices
  (stationary) and Wv chunks (moving) -- no separate vT projection or
  V re-transpose. Its bias folds into the output bias:
  O = A_norm(xWv)Wo + (bo + bv@Wo), since A is row-normalized; the
  two degenerate bv@Wo matmuls run once in build_bo_bc.
- Attention slots for q-chunk 0 are interleaved into the projection
  phase (4 slots after each 4-stile group); chunks 1-3 follow as ONE
  flat slot stream: slot (qq,t) emits score+exp, the H/rowsum of the
  slot LOOK=3 behind (carried across chunk boundaries), and one
  stuffed out-projection of the previous chunk per 2 slots.
- Rowsum stationary is a full [128,128] ones matrix: M=1 stationaries
  pay ~+93 ns on entry AND exit (degenerate-shape pipeline flush),
  and M=128 writes the rowsum pre-broadcast across partitions so the
  reciprocal needs no separate broadcast matmul.
- p tiles are pre-summed on the vector engine in two levels (pairs,
  then quads; bf16 adds) so the rowsum matmul runs once per FOUR
  tiles. The final chunk's last four tiles use direct per-tile
  rowsums so the kernel tail does not wait on the DVE pre-sum chain.
- Drain-phase out-projections rotate PSUM banks across the kqv and s
  rings so the matmuls pipeline ahead of their DVE bias-adds.
- Every accumulation step that interleaves with other banks carries
  stop=True (sim-only flag; start=False still accumulates): groups
  left open across bank switches cost ~+93 ns per transition.
- PSUM budget (8 banks): tp 1 + kqv 2 + s 2 + h 2 + r 1.
- Rejected with measurements: fp8 anywhere on the value path (3-bit
  mantissa -> ~5% output rel err vs 2e-2 budget; random-sign sums
  give NO sqrt(N) error averaging), PE filler/warm-up matmuls (the
  chip's utilization governor then throttles EVERY engine ~20% for
  the rest of the kernel), dtype-converting DVE copies in the hot
  path (lower to CAST instructions, ~400ns fixed cost each),
  scalar_tensor_tensor for the normalize fold (895ns vs 690ns
  tensor_add), gpsimd for any PSUM traffic (no PSUM port), DVE
  StreamTranspose for x^T (32x32 blocks only).
"""

import sys
from contextlib import ExitStack

for _p in ("/root/.axon_site", "/root/.axon_site/_ro/trn_rl_repo", "/opt/trn_rl_repo"):
    if _p not in sys.path:
        sys.path.append(_p)

import numpy as np

B, S, E = 8, 2048, 1024
KQ = 128
N_CORES = 8
S_TILES = S // 128          # 16
E_CHUNKS = E // 128         # 8
Q_CHUNK = 512               # q columns processed per attention pass
N_QCHUNKS = S // Q_CHUNK    # 4
SCALE = float(1.0 / np.sqrt(KQ))
LOOK = 3                    # score-tile lookahead ahead of H matmuls


def build_nc():
    import concourse.bass as bass
    import concourse.tile as tile
    from concourse import bacc, mybir
    from concourse.masks import make_identity

    f32 = mybir.dt.float32
    bf16 = mybir.dt.bfloat16
    Exp = mybir.ActivationFunctionType.Exp

    nc = bacc.Bacc("TRN2", target_bir_lowering=False, debug=False,
                   num_devices=N_CORES)

    x_ext = nc.declare_dram_parameter("x", [S, E], f32, isOutput=False)
    wq_ext = nc.declare_dram_parameter("Wq", [E, KQ], f32, isOutput=False)
    bq_ext = nc.declare_dram_parameter("bq", [KQ], f32, isOutput=False)
    wk_ext = nc.declare_dram_parameter("Wk", [E, KQ], f32, isOutput=False)
    bk_ext = nc.declare_dram_parameter("bk", [KQ], f32, isOutput=False)
    wv_ext = nc.declare_dram_parameter("Wv", [E, KQ], f32, isOutput=False)
    bv_ext = nc.declare_dram_parameter("bv", [KQ], f32, isOutput=False)
    wo_ext = nc.declare_dram_parameter("Wo", [KQ, E], f32, isOutput=False)
    bo_ext = nc.declare_dram_parameter("bo", [E], f32, isOutput=False)
    out_ext = nc.declare_dram_parameter("out", [S, E], f32, isOutput=True)

    with tile.TileContext(nc) as tc, ExitStack() as ctx:
        singles = ctx.enter_context(tc.tile_pool(name="singles", bufs=1))
        xb_pool = ctx.enter_context(tc.tile_pool(name="xb", bufs=5))
        pt_pool = ctx.enter_context(tc.tile_pool(name="pt", bufs=12))
        rs_pool = ctx.enter_context(tc.tile_pool(name="rs", bufs=2))
        o_pool = ctx.enter_context(tc.tile_pool(name="o", bufs=4))
        # PSUM budget (8 banks of [128,2KB]): tp 1 + kqv 2 + s 2 + h 2
        # + r 1 = 8. tp is ONE [128,1024]bf16 bank (8 transposes of a
        # stile); kqv is a 2-ring shared by the K/Q/V projection
        # accumulators, bo build, and the out-projections.
        ps_mm = ctx.enter_context(tc.tile_pool(name="ps_mm", bufs=2, space="PSUM"))
        ps_s = ctx.enter_context(tc.tile_pool(name="ps_s", bufs=2, space="PSUM"))
        ps_h = ctx.enter_context(tc.tile_pool(name="ps_h", bufs=2, space="PSUM"))
        ps_r = ctx.enter_context(tc.tile_pool(name="ps_r", bufs=1, space="PSUM"))

        # ---- tiny constants first (transposes need ident) ------------
        ones_row = singles.tile([1, 128], bf16)
        nc.vector.memset(ones_row[:], 1.0)
        # full ones matrix: rowsum matmuls with M=128 stationary write
        # the rowsum broadcast across all 128 partitions (M=1 costs
        # ~+93 ns on entry and exit of every rowsum matmul).
        ones_mat = singles.tile([128, 128], bf16)
        nc.vector.memset(ones_mat[:], 1.0)
        ident = singles.tile([128, 128], bf16)


        # ---- ONE gpsimd cast-DMA stream: x tiles + weights in PE -----
        # consumption order. Weight tensors each load in a single op
        # ([E,KQ] f32 -> [128, E] bf16 chunk-major).
        wq_t = singles.tile([128, E], bf16)   # chunk j at [:, 128j:128j+128]
        wk_t = singles.tile([128, E], bf16)
        wv_t = singles.tile([128, E], bf16)
        wo_t = singles.tile([128, E], bf16)   # [v, e]

        def load_w(w_t, w_ext):
            nc.gpsimd.dma_start(
                out=w_t[:].rearrange("p (j c) -> p j c", j=E_CHUNKS),
                in_=w_ext[:].rearrange("(j p) c -> p j c", p=128))

        def load_wo():
            nc.gpsimd.dma_start(out=wo_t[:], in_=wo_ext[:])

        # tiny warm-up read: absorbs the ~3 us first-transfer latency
        # of the gpsimd DMA queue before the real x stream begins
        warm = singles.tile([1, 32], f32)
        nc.gpsimd.dma_start(out=warm[:], in_=x_ext[0:1, 0:32])

        xb_tiles = []          # (tile, first_stile, n_stiles)

        def load_x(first, nst, halves=1):
            xbt = xb_pool.tile([128, nst, E], bf16, tag="xb",
                               name=f"xb{first}")
            src = x_ext[first * 128:(first + nst) * 128, :].rearrange(
                "(c p) e -> p c e", p=128)
            # halves=2 splits the op along E: the transpose groups
            # consume e-chunks 0-3 and 4-7 separately (sub-tile deps),
            # so the first PE work starts one half-transfer earlier.
            eh = E // halves
            for h in range(halves):
                nc.gpsimd.dma_start(           # cast f32 -> bf16 in DMA
                    out=xbt[:, :, h * eh:(h + 1) * eh],
                    in_=src[:, :, h * eh:(h + 1) * eh])
            xb_tiles.append((xbt, first, nst))

        load_x(0, 1, halves=2)
        load_x(1, 1)
        # ident built after the first x descriptors are issued: its
        # iota/affine_select ucode otherwise delays the x stream start
        # by ~0.6 us on the gpsimd engine
        make_identity(nc, ident[:])
        load_w(wk_t, wk_ext)
        load_x(2, 2)
        load_w(wv_t, wv_ext)
        # wq directly after wv: qT chunk 0 projects at the end of
        # group 0 (~18us) and chunk-0 scores follow right behind.
        load_w(wq_t, wq_ext)
        load_x(4, 2)
        load_x(6, 2)
        load_x(8, 2)
        load_wo()
        load_x(10, 2)
        load_x(12, 2)
        load_x(14, 2)

        def xb_stile(i):
            """SBUF AP of x s-tile i: [128, E] bf16."""
            for xbt, first, nst in xb_tiles:
                if first <= i < first + nst:
                    return xbt[:, i - first, :]
            raise IndexError(i)

        # ---- biases on the sync queue (tiny, f32, no cast) -----------
        bq_t = singles.tile([128, 1], f32)
        nc.sync.dma_start(out=bq_t[:], in_=bq_ext[:])
        bk_t = singles.tile([128, 1], f32)
        nc.sync.dma_start(out=bk_t[:], in_=bk_ext[:])
        bv_t = singles.tile([128, 1], f32)
        nc.sync.dma_start(out=bv_t[:], in_=bv_ext[:])
        bo_row = singles.tile([1, E], f32)
        nc.sync.dma_start(out=bo_row[:], in_=bo_ext[:].rearrange("(o e) -> o e", o=1))
        bv_col16 = singles.tile([128, 1], bf16)
        bo_eff16 = singles.tile([1, E], bf16)
        bo_bc = singles.tile([128, E], f32)

        def build_bo_bc():
            # V's bias is folded into an effective output bias:
            #   O = softmax(..) (xWv) Wo + (bo + bv @ Wo)
            # (A is row-normalized, so the +bv term of V contributes
            # exactly bv @ Wo to every output row.) bo_eff is then
            # broadcast across partitions with a K=1 outer product.
            # Emitted AFTER the projection groups: needs wo_t, and at
            # the head of the PE program it inherits a DMA-queue wait
            # that stalls all PE work.
            nc.vector.tensor_copy(bv_col16[:], bv_t[:])
            for half in range(2):
                hs = slice(half * 512, (half + 1) * 512)
                bvwo_ps = ps_mm.tile([128, 512], f32, tag="kqv", bufs=2,
                                     name=f"bvwo{half}")
                nc.tensor.matmul(bvwo_ps[0:1, :], bv_col16[:], wo_t[:, hs],
                                 start=True, stop=True)
                nc.vector.tensor_add(bo_eff16[:, hs], bvwo_ps[0:1, :],
                                     bo_row[:, hs])
                bo_ps = ps_mm.tile([128, 512], f32, tag="kqv", bufs=2,
                                     name=f"bo{half}")
                nc.tensor.matmul(bo_ps[:], ones_row[:], bo_eff16[:, hs],
                                 start=True, stop=True)
                nc.vector.tensor_copy(bo_bc[:, hs], bo_ps[:])

        # ---- x^T via TensorE transposes ------------------------------
        # xT_big[:, j*S + s] = x[s, j*128 + p]  (e-chunk j on partitions)
        xT_big = singles.tile([128, E_CHUNKS * S], bf16)
        xT = xT_big[:].rearrange("p (j s) -> p j s", j=E_CHUNKS)

        def transpose_stile(i):
            src = xb_stile(i)
            # all 8 transposes of a stile fit ONE 2KB bank in bf16
            tp_ps = ps_mm.tile([128, 1024], bf16, tag="tp", bufs=1,
                               name=f"tp{i}")
            for j in range(E_CHUNKS):
                nc.tensor.transpose(
                    tp_ps[:, j * 128:(j + 1) * 128],
                    src[:, j * 128:(j + 1) * 128],
                    ident[:])
            for jh in range(2):   # two half copies: finer consumer deps
                nc.vector.tensor_copy(
                    xT[:, jh * 4:(jh + 1) * 4, i * 128:(i + 1) * 128],
                    tp_ps[:, jh * 512:(jh + 1) * 512].rearrange(
                        "p (j s) -> p j s", j=4))

        # ---- projections: K^T, Q^T [d, S]; V natural -----------------
        qT = singles.tile([128, S], bf16)
        kT = singles.tile([128, S], bf16)
        v_big = singles.tile([128, S], bf16)   # k-tile t at [:, 128t:128t+128]

        # K/Q projections run PER-STILE (8x 128-col matmuls each, the
        # xT half-copies as inputs) so projection work interleaves with
        # the transposes and fills the DMA-paced gaps at the head --
        # which also keeps the PE p-state ramp alive.
        kq_banks = {}

        def proj_stile(i):
            c = i // 4
            if ("k", c) not in kq_banks:
                kq_banks[("k", c)] = ps_mm.tile(
                    [128, 512], f32, tag="kqv", bufs=2, name=f"kbank{c}")
                kq_banks[("q", c)] = ps_mm.tile(
                    [128, 512], f32, tag="kqv", bufs=2, name=f"qbank{c}")
            sl = slice((i % 4) * 128, (i % 4 + 1) * 128)
            # half-interleaved: all e-chunks 0-3 (first xT half-copy)
            # for BOTH K and Q before touching chunks 4-7, so the
            # 0-behind head stiles never stall on the second half-copy
            for jh in range(2):
                for key, w_t in (("k", wk_t), ("q", wq_t)):
                    bank = kq_banks[(key, c)]
                    for jj in range(4):
                        j = jh * 4 + jj
                        # stop=True at each 4-run end: sim-only flag;
                        # leaving the group open across the K/Q bank
                        # switches costs ~93ns per transition
                        nc.tensor.matmul(
                            bank[:, sl],
                            w_t[:, j * 128:(j + 1) * 128],
                            xT[:, j, i * 128:(i + 1) * 128],
                            start=(j == 0), stop=(jj == 3),
                            skip_group_check=True)

        def vnatural_group(c):
            # V in NATURAL [s(k), v] layout, one 128-col matmul per
            # (s-tile, e-chunk) with the xT slice as stationary:
            # out = x_stile @ Wv_chunk. Replaces the vT projection +
            # PE re-transpose; bias bv is folded into bo_eff instead
            # (see build_bo_bc), so no scalar add either.
            vp_ps = ps_mm.tile([128, 512], f32, tag="kqv", bufs=2,
                               name=f"vn{c}")
            for tt in range(4):
                i = c * 4 + tt
                for j in range(E_CHUNKS):
                    nc.tensor.matmul(
                        vp_ps[:, tt * 128:(tt + 1) * 128],
                        xT[:, j, i * 128:(i + 1) * 128],
                        wv_t[:, j * 128:(j + 1) * 128],
                        start=(j == 0), stop=(j == E_CHUNKS - 1))
            nc.vector.tensor_copy(
                v_big[:, c * 512:(c + 1) * 512], vp_ps[:])

        # ---- attention machinery (flat slot stream) ------------------
        hT = singles.tile([128, S], bf16)      # normalized H^T [v, q]

        outproj_q = []                         # out-proj closures
        pending_hr = []                        # (qq, t) H/rowsum slots
        chunk_state = {}                       # qq -> (h_ps, r_ps, p_ts, pq_ts, pquad_ts)

        def make_outproj(s0, half):
            def emit(pool=ps_mm, tag="kqv", queue=None):
                o_ps = pool.tile([128, 512], f32, tag=tag, bufs=2)
                nc.tensor.matmul(o_ps[:],
                                 hT[:, s0:s0 + 128],
                                 wo_t[:, half * 512:(half + 1) * 512],
                                 start=True, stop=True)
                o_sb = o_pool.tile([128, 512], f32, tag="o_sb")
                nc.vector.tensor_add(
                    o_sb[:], o_ps[:],
                    bo_bc[:, half * 512:(half + 1) * 512])
                (queue or nc.sync).dma_start(
                    out=out_ext[s0:s0 + 128,
                                half * 512:(half + 1) * 512],
                    in_=o_sb[:])
            return emit

        def emit_hr(qq, t):
            h_ps, r_ps, p_ts, pq_ts, pquad_ts = chunk_state[qq]
            # stop=True on every accumulation step: leaving the group
            # open across interleaved bank switches costs ~93 ns per
            # transition (PSUM pipeline flush); stop is sim-bookkeeping
            # only, accumulation continues via start=False.
            nc.tensor.matmul(h_ps[:], v_big[:, t * 128:(t + 1) * 128],
                             p_ts[t // 2][:, t % 2, :],
                             start=(t == 0), stop=True,
                             skip_group_check=True)
            if qq == N_QCHUNKS - 1 and t >= S_TILES - 4:
                # final four tiles of the final chunk: direct per-tile
                # rowsums so the normalize does not wait on the DVE
                # pre-sum chain at the very tail of the kernel
                nc.tensor.matmul(r_ps[:], ones_mat[:],
                                 p_ts[t // 2][:, t % 2, :],
                                 start=False, stop=True,
                                 skip_group_check=True)
            elif t % 2 == 1:
                # rowsum of a vector-presummed p-tile PAIR. (Pairs,
                # not quads: the mid-phase pacer is the SATURATED
                # vector engine, so the extra quad-level DVE add costs
                # more than the rowsum matmul it saves on the PE.)
                nc.tensor.matmul(r_ps[:], ones_mat[:], pq_ts[t // 2][:],
                                 start=(t == 1), stop=True,
                                 skip_group_check=True)
            if t == S_TILES - 1:
                finish_chunk(qq)

        def finish_chunk(qq):
            h_ps, r_ps, p_ts, pq_ts, pquad_ts = chunk_state[qq]
            qs = qq * Q_CHUNK
            # rowsum already broadcast across partitions; reciprocal
            # directly on the [128, Q_CHUNK] PSUM bank.
            r_bc = rs_pool.tile([128, Q_CHUNK], f32, tag="r_bc")
            nc.vector.reciprocal_approx_fast(r_bc[:], r_ps[:])
            # ONE [128,512] normalize mul (the DVE is the mid-phase
            # pacer; 1x ~690ns beats 4x ~500ns per-si muls)
            nc.vector.tensor_mul(hT[:, qs:qs + Q_CHUNK], h_ps[:], r_bc[:])
            for si in range(Q_CHUNK // 128):
                for half in range(2):
                    outproj_q.append(make_outproj(qs + si * 128, half))

        def emit_slot(qq, t):
            qs = qq * Q_CHUNK
            final_tail = (qq == N_QCHUNKS - 1 and t >= S_TILES - 4)
            s_ps = ps_s.tile([128, Q_CHUNK], f32, tag="s")
            nc.tensor.matmul(s_ps[:],
                             kT[:, t * 128:(t + 1) * 128],
                             qT[:, qs:qs + Q_CHUNK],
                             start=True, stop=True)
            if t % 2 == 0:
                pp = pt_pool.tile([128, 2, Q_CHUNK], bf16, tag="p",
                                  name=f"p{qq}_{t // 2}")
                chunk_state[qq][2].append(pp)
            p_t = chunk_state[qq][2][t // 2][:, t % 2, :]
            nc.scalar.activation(out=p_t, in_=s_ps[:], func=Exp,
                                 scale=SCALE)
            if t % 2 == 1 and not final_tail:
                # vector pre-sum of the p pair (bf16, ~350 ns)
                pq = pt_pool.tile([128, Q_CHUNK], bf16, tag="pq",
                                  name=f"pq{qq}_{t // 2}")
                chunk_state[qq][3].append(pq)
                # pair pre-sum on GPSIMD: it is idle all through the
                # attention phase while the DVE is the saturated
                # pacer (o_sb adds + normalize + these). SBUF-only
                # bf16 add, so no PSUM port needed.
                nc.gpsimd.tensor_add(
                    pq[:], chunk_state[qq][2][t // 2][:, 0, :],
                    chunk_state[qq][2][t // 2][:, 1, :])
            pending_hr.append((qq, t))
            if len(pending_hr) > LOOK:
                emit_hr(*pending_hr.pop(0))
            # out-projections of the previous chunk on the slots
            # whose H is even (no rowsum there); hold off until
            # slot 5 (its normalize lands ~slot 4) unless backlogged
            if outproj_q and ((t % 2 == 1 and t >= 5)
                              or len(outproj_q) > 2):
                outproj_q.pop(0)()

        def open_chunk(qq):
            h_ps = ps_h.tile([128, Q_CHUNK], f32, tag="h")
            r_ps = ps_r.tile([128, Q_CHUNK], f32, tag="r")
            chunk_state[qq] = (h_ps, r_ps, [], [], [])

        def finish_group(c):
            # chunk c's K/Q banks are fully accumulated: bias-add them
            # out (scalar), compute the group's natural-V tiles, then
            # emit the four chunk-0 attention slots this group unblocks
            nc.scalar.add(kT[:, c * 512:(c + 1) * 512],
                          kq_banks[("k", c)][:], bk_t[:])
            nc.scalar.add(qT[:, c * 512:(c + 1) * 512],
                          kq_banks[("q", c)][:], bq_t[:])
            vnatural_group(c)
            for t in range(4 * c, 4 * c + 4):
                emit_slot(0, t)

        # ---- stile pipeline: transposes + per-stile K/Q projections --
        # Stiles 0-3 project 0-behind (the head is DMA-paced, so the
        # projection matmuls fill the x-arrival gaps and keep the PE
        # p-state ramp alive); stiles 4+ project 1-behind their
        # transpose so the xT half-copies (DVE) are never waited on.
        open_chunk(0)
        for i in range(S_TILES):
            transpose_stile(i)
            pi = i if i < 4 else (i - 1 if i >= 5 else None)
            if pi is not None:
                proj_stile(pi)
                if pi % 4 == 3:
                    finish_group(pi // 4)
        proj_stile(S_TILES - 1)
        finish_group(N_QCHUNKS - 1)

        build_bo_bc()   # needs wo_t (landed ~35us); first use ~45us

        # ---- remaining attention chunks ------------------------------
        for qq in range(1, N_QCHUNKS):
            open_chunk(qq)
            for t in range(S_TILES):
                emit_slot(qq, t)

        while pending_hr:
            emit_hr(*pending_hr.pop(0))
        # drain: rotate o_ps across ps_mm + ps_s (5 PSUM banks) so the
        # matmuls pipeline ahead of their DVE consumers
        # drain: rotate PSUM rings AND alternate the two DMA queues
        # (gpsimd is idle at the tail) so the final writes drain in
        # parallel
        drain_pools = [(ps_mm, "kqv"), (ps_s, "s")]
        di = 0
        while outproj_q:
            pool, tag = drain_pools[di % len(drain_pools)]
            outproj_q.pop(0)(pool, tag,
                             nc.gpsimd if di % 2 else nc.sync)
            di += 1

    nc.compile()
    return nc


_NC = None


def kernel(**inputs):
    global _NC
    from concourse.bass_utils import run_bass_kernel_spmd

    if _NC is None:
        _NC = build_nc()

    x = np.asarray(inputs["embedding_matrix"], dtype=np.float32)
    shared = {k: np.ascontiguousarray(np.asarray(inputs[k], dtype=np.float32))
              for k in ("Wq", "bq", "Wk", "bk", "Wv", "bv", "Wo", "bo")}
    in_maps = [dict(shared, x=np.ascontiguousarray(x[c])) for c in range(N_CORES)]

    res = run_bass_kernel_spmd(_NC, in_maps, core_ids=list(range(N_CORES)))
    out = np.stack([res.results[c]["out"] for c in range(N_CORES)], axis=0)
    return out.astype(np.float32)


# revision 46
# speedup vs baseline: 1.0102x; 1.0102x over previous
"""Single-head attention layer on 8 TRN2 NeuronCores.

Data-parallel over batch: B=8 batch elements, one per core. Each core
computes, for its x [S=2048, E=1024] slice:
    Q = x@Wq+bq; K = x@Wk+bk; V = x@Wv+bv        (KQ = VDIM = 128)
    O = softmax(Q K^T / sqrt(128)) V @ Wo + bo
All matmuls run in bf16 with fp32 PSUM accumulation (measured L2 rel
err ~1e-3 vs the f32 reference). Softmax skips the max-subtraction
(scores are in [-2.5, 2.5] for this input distribution) so the row sum
can be computed with a ones-vector matmul and normalization folds into
the H^T PSUM->SBUF copy.

Perf notes (trace-driven; 137us -> ~110us; measured exec has +-2.5us
run-to-run noise, so treat single samples with care):
- Measured PE cadence model: MM duration = ~165ns + cols*0.42ns, but
  back-to-back MMs PIPELINE: sustained spacing 216ns (512-col) /
  56ns (128-col) at full clock. The PE runs at HALF clock until ~3us
  of CONTINUOUS execution (gaps reset the ramp; full speed lands
  ~24.5us because the early x arrivals trickle). Consecutive scalar
  exps pipeline at ~570ns despite 686ns durations.
- ALL HBM reads ride ONE gpsimd cast-DMA stream in PE-consumption
  order (x tiles and weights interleaved; each W one DMA op). One
  queue saturates the ~350 GB/s per-core HBM port. DMA-op completion
  semaphores fire in ISSUE order with multi-us lag on the first few
  ops -- reordering early loads to "arrive later but denser"
  backfires. A tiny warm-up read absorbs first-transfer latency.
- The first matmul in PE program order inherits a spurious wait on
  the gpsimd DMA-queue op semaphore (fires ~8.6-16us); anything
  data-independent (bo broadcast) must NOT sit at the PE head.
- x^T is built per-stile on the PE (8 transposes into ONE
  [128,1024]bf16 bank = one 2KB PSUM bank), and the K/Q projections
  run PER-STILE (8x 128-col MMs each, half-interleaved K/Q so the
  second xT half-copy is never waited on). Stiles 0-3 project
  0-behind their transpose to fill the DMA-paced head gaps; stiles
  4+ project 1-behind so the DVE xT copies are fully hidden.
- V is computed directly in NATURAL [s,v] layout from the xT slices
  (stationary) and Wv chunks (moving) -- no separate vT projection or
  V re-transpose. Its bias folds into the output bias:
  O = A_norm(xWv)Wo + (bo + bv@Wo), since A is row-normalized; the
  two degenerate bv@Wo matmuls run once in build_bo_bc.
- Attention slots for q-chunk 0 are interleaved into the projection
  phase (4 slots after each 4-stile group); chunks 1-3 follow as ONE
  flat slot stream: slot (qq,t) emits score+exp, the H/rowsum of the
  slot LOOK=3 behind (carried across chunk boundaries), and one
  stuffed out-projection of the previous chunk per 2 slots.
- Rowsum stationary is a full [128,128] ones matrix: M=1 stationaries
  pay ~+93 ns on entry AND exit (degenerate-shape pipeline flush),
  and M=128 writes the rowsum pre-broadcast across partitions so the
  reciprocal needs no separate broadcast matmul.
- p tiles are pre-summed in PAIRS on the vector engine so the rowsum
  matmul runs once per two tiles. NOT quads: the mid-phase pacer is
  the saturated DVE (pair adds + o_sb bias adds + normalize ~10.2us
  vs PE ~10.4us per 10.8us chunk), so a deeper pre-sum tree trades
  cheap PE time for expensive DVE time. The final chunk's last four
  tiles use direct per-tile rowsums so the kernel tail does not wait
  on the DVE pre-sum chain. The normalize is ONE [128,512] mul, not
  four per-si muls, for the same reason.
- Drain-phase out-projections rotate PSUM banks across the kqv and s
  rings so the matmuls pipeline ahead of their DVE bias-adds, and
  alternate the sync/gpsimd DMA queues so the final writes drain in
  parallel.
- Every accumulation step that interleaves with other banks carries
  stop=True (sim-only flag; start=False still accumulates): groups
  left open across bank switches cost ~+93 ns per transition.
- PSUM budget (8 banks): tp 1 + kqv 2 + s 2 + h 2 + r 1.
- Rejected with measurements: fp8 anywhere on the value path (3-bit
  mantissa -> ~5% output rel err vs 2e-2 budget; random-sign sums
  give NO sqrt(N) error averaging), PE filler/warm-up matmuls (the
  chip's utilization governor then throttles EVERY engine ~20% for
  the rest of the kernel), dtype-converting DVE copies in the hot
  path (lower to CAST instructions, ~400ns fixed cost each),
  scalar_tensor_tensor for the normalize fold (895ns vs 690ns
  tensor_add), gpsimd for any PSUM traffic (no PSUM port), DVE
  StreamTranspose for x^T (32x32 blocks only).
"""

import sys
from contextlib import ExitStack

for _p in ("/root/.axon_site", "/root/.axon_site/_ro/trn_rl_repo", "/opt/trn_rl_repo"):
    if _p not in sys.path:
        sys.path.append(_p)

import numpy as np

B, S, E = 8, 2048, 1024
KQ = 128
N_CORES = 8
S_TILES = S // 128          # 16
E_CHUNKS = E // 128         # 8
Q_CHUNK = 512               # q columns processed per attention pass
N_QCHUNKS = S // Q_CHUNK    # 4
SCALE = float(1.0 / np.sqrt(KQ))
LOOK = 3                    # score-tile lookahead ahead of H matmuls


def build_nc():
    import concourse.bass as bass
    import concourse.tile as tile
    from concourse import bacc, mybir
    from concourse.masks import make_identity

    f32 = mybir.dt.float32
    bf16 = mybir.dt.bfloat16
    Exp = mybir.ActivationFunctionType.Exp

    nc = bacc.Bacc("TRN2", target_bir_lowering=False, debug=False,
                   num_devices=N_CORES)

    x_ext = nc.declare_dram_parameter("x", [S, E], f32, isOutput=False)
    wq_ext = nc.declare_dram_parameter("Wq", [E, KQ], f32, isOutput=False)
    bq_ext = nc.declare_dram_parameter("bq", [KQ], f32, isOutput=False)
    wk_ext = nc.declare_dram_parameter("Wk", [E, KQ], f32, isOutput=False)
    bk_ext = nc.declare_dram_parameter("bk", [KQ], f32, isOutput=False)
    wv_ext = nc.declare_dram_parameter("Wv", [E, KQ], f32, isOutput=False)
    bv_ext = nc.declare_dram_parameter("bv", [KQ], f32, isOutput=False)
    wo_ext = nc.declare_dram_parameter("Wo", [KQ, E], f32, isOutput=False)
    bo_ext = nc.declare_dram_parameter("bo", [E], f32, isOutput=False)
    out_ext = nc.declare_dram_parameter("out", [S, E], f32, isOutput=True)

    with tile.TileContext(nc) as tc, ExitStack() as ctx:
        singles = ctx.enter_context(tc.tile_pool(name="singles", bufs=1))
        xb_pool = ctx.enter_context(tc.tile_pool(name="xb", bufs=5))
        pt_pool = ctx.enter_context(tc.tile_pool(name="pt", bufs=12))
        rs_pool = ctx.enter_context(tc.tile_pool(name="rs", bufs=2))
        o_pool = ctx.enter_context(tc.tile_pool(name="o", bufs=4))
        # PSUM budget (8 banks of [128,2KB]): tp 1 + kqv 2 + s 2 + h 2
        # + r 1 = 8. tp is ONE [128,1024]bf16 bank (8 transposes of a
        # stile); kqv is a 2-ring shared by the K/Q/V projection
        # accumulators, bo build, and the out-projections.
        ps_mm = ctx.enter_context(tc.tile_pool(name="ps_mm", bufs=2, space="PSUM"))
        ps_s = ctx.enter_context(tc.tile_pool(name="ps_s", bufs=2, space="PSUM"))
        ps_h = ctx.enter_context(tc.tile_pool(name="ps_h", bufs=2, space="PSUM"))
        ps_r = ctx.enter_context(tc.tile_pool(name="ps_r", bufs=1, space="PSUM"))

        # ---- tiny constants first (transposes need ident) ------------
        ones_row = singles.tile([1, 128], bf16)
        nc.vector.memset(ones_row[:], 1.0)
        # full ones matrix: rowsum matmuls with M=128 stationary write
        # the rowsum broadcast across all 128 partitions (M=1 costs
        # ~+93 ns on entry and exit of every rowsum matmul).
        ones_mat = singles.tile([128, 128], bf16)
        nc.vector.memset(ones_mat[:], 1.0)
        ident = singles.tile([128, 128], bf16)


        # ---- ONE gpsimd cast-DMA stream: x tiles + weights in PE -----
        # consumption order. Weight tensors each load in a single op
        # ([E,KQ] f32 -> [128, E] bf16 chunk-major).
        wq_t = singles.tile([128, E], bf16)   # chunk j at [:, 128j:128j+128]
        wk_t = singles.tile([128, E], bf16)
        wv_t = singles.tile([128, E], bf16)
        wo_t = singles.tile([128, E], bf16)   # [v, e]

        def load_w(w_t, w_ext):
            nc.gpsimd.dma_start(
                out=w_t[:].rearrange("p (j c) -> p j c", j=E_CHUNKS),
                in_=w_ext[:].rearrange("(j p) c -> p j c", p=128))

        def load_wo():
            nc.gpsimd.dma_start(out=wo_t[:], in_=wo_ext[:])

        # tiny warm-up read: absorbs the ~3 us first-transfer latency
        # of the gpsimd DMA queue before the real x stream begins
        warm = singles.tile([1, 32], f32)
        nc.gpsimd.dma_start(out=warm[:], in_=x_ext[0:1, 0:32])

        xb_tiles = []          # (tile, first_stile, n_stiles)

        def load_x(first, nst, halves=1):
            xbt = xb_pool.tile([128, nst, E], bf16, tag="xb",
                               name=f"xb{first}")
            src = x_ext[first * 128:(first + nst) * 128, :].rearrange(
                "(c p) e -> p c e", p=128)
            # halves=2 splits the op along E: the transpose groups
            # consume e-chunks 0-3 and 4-7 separately (sub-tile deps),
            # so the first PE work starts one half-transfer earlier.
            eh = E // halves
            for h in range(halves):
                nc.gpsimd.dma_start(           # cast f32 -> bf16 in DMA
                    out=xbt[:, :, h * eh:(h + 1) * eh],
                    in_=src[:, :, h * eh:(h + 1) * eh])
            xb_tiles.append((xbt, first, nst))

        load_x(0, 1, halves=2)
        load_x(1, 1)
        # ident built after the first x descriptors are issued: its
        # iota/affine_select ucode otherwise delays the x stream start
        # by ~0.6 us on the gpsimd engine
        make_identity(nc, ident[:])
        load_w(wk_t, wk_ext)
        load_x(2, 2)
        load_w(wv_t, wv_ext)
        # wq directly after wv: qT chunk 0 projects at the end of
        # group 0 (~18us) and chunk-0 scores follow right behind.
        load_w(wq_t, wq_ext)
        load_x(4, 2)
        load_x(6, 2)
        load_x(8, 2)
        load_wo()
        load_x(10, 2)
        load_x(12, 2)
        load_x(14, 2)

        def xb_stile(i):
            """SBUF AP of x s-tile i: [128, E] bf16."""
            for xbt, first, nst in xb_tiles:
                if first <= i < first + nst:
                    return xbt[:, i - first, :]
            raise IndexError(i)

        # ---- biases on the sync queue (tiny, f32, no cast) -----------
        bq_t = singles.tile([128, 1], f32)
        nc.sync.dma_start(out=bq_t[:], in_=bq_ext[:])
        bk_t = singles.tile([128, 1], f32)
        nc.sync.dma_start(out=bk_t[:], in_=bk_ext[:])
        bv_t = singles.tile([128, 1], f32)
        nc.sync.dma_start(out=bv_t[:], in_=bv_ext[:])
        bo_row = singles.tile([1, E], f32)
        nc.sync.dma_start(out=bo_row[:], in_=bo_ext[:].rearrange("(o e) -> o e", o=1))
        bv_col16 = singles.tile([128, 1], bf16)
        bo_eff16 = singles.tile([1, E], bf16)
        bo_bc = singles.tile([128, E], f32)

        def build_bo_bc():
            # V's bias is folded into an effective output bias:
            #   O = softmax(..) (xWv) Wo + (bo + bv @ Wo)
            # (A is row-normalized, so the +bv term of V contributes
            # exactly bv @ Wo to every output row.) bo_eff is then
            # broadcast across partitions with a K=1 outer product.
            # Emitted AFTER the projection groups: needs wo_t, and at
            # the head of the PE program it inherits a DMA-queue wait
            # that stalls all PE work.
            nc.vector.tensor_copy(bv_col16[:], bv_t[:])
            for half in range(2):
                hs = slice(half * 512, (half + 1) * 512)
                bvwo_ps = ps_mm.tile([128, 512], f32, tag="kqv", bufs=2,
                                     name=f"bvwo{half}")
                nc.tensor.matmul(bvwo_ps[0:1, :], bv_col16[:], wo_t[:, hs],
                                 start=True, stop=True)
                nc.vector.tensor_add(bo_eff16[:, hs], bvwo_ps[0:1, :],
                                     bo_row[:, hs])
                bo_ps = ps_mm.tile([128, 512], f32, tag="kqv", bufs=2,
                                     name=f"bo{half}")
                nc.tensor.matmul(bo_ps[:], ones_row[:], bo_eff16[:, hs],
                                 start=True, stop=True)
                nc.vector.tensor_copy(bo_bc[:, hs], bo_ps[:])

        # ---- x^T via TensorE transposes ------------------------------
        # xT_big[:, j*S + s] = x[s, j*128 + p]  (e-chunk j on partitions)
        xT_big = singles.tile([128, E_CHUNKS * S], bf16)
        xT = xT_big[:].rearrange("p (j s) -> p j s", j=E_CHUNKS)

        def transpose_stile(i):
            src = xb_stile(i)
            # all 8 transposes of a stile fit ONE 2KB bank in bf16
            tp_ps = ps_mm.tile([128, 1024], bf16, tag="tp", bufs=1,
                               name=f"tp{i}")
            for j in range(E_CHUNKS):
                nc.tensor.transpose(
                    tp_ps[:, j * 128:(j + 1) * 128],
                    src[:, j * 128:(j + 1) * 128],
                    ident[:])
            for jh in range(2):   # two half copies: finer consumer deps
                nc.vector.tensor_copy(
                    xT[:, jh * 4:(jh + 1) * 4, i * 128:(i + 1) * 128],
                    tp_ps[:, jh * 512:(jh + 1) * 512].rearrange(
                        "p (j s) -> p j s", j=4))

        # ---- projections: K^T, Q^T [d, S]; V natural -----------------
        qT = singles.tile([128, S], bf16)
        kT = singles.tile([128, S], bf16)
        v_big = singles.tile([128, S], bf16)   # k-tile t at [:, 128t:128t+128]

        # K/Q projections run PER-STILE (8x 128-col matmuls each, the
        # xT half-copies as inputs) so projection work interleaves with
        # the transposes and fills the DMA-paced gaps at the head --
        # which also keeps the PE p-state ramp alive.
        kq_banks = {}

        def proj_stile(i):
            c = i // 4
            if ("k", c) not in kq_banks:
                kq_banks[("k", c)] = ps_mm.tile(
                    [128, 512], f32, tag="kqv", bufs=2, name=f"kbank{c}")
                kq_banks[("q", c)] = ps_mm.tile(
                    [128, 512], f32, tag="kqv", bufs=2, name=f"qbank{c}")
            sl = slice((i % 4) * 128, (i % 4 + 1) * 128)
            # half-interleaved: all e-chunks 0-3 (first xT half-copy)
            # for BOTH K and Q before touching chunks 4-7, so the
            # 0-behind head stiles never stall on the second half-copy
            for jh in range(2):
                for key, w_t in (("k", wk_t), ("q", wq_t)):
                    bank = kq_banks[(key, c)]
                    for jj in range(4):
                        j = jh * 4 + jj
                        # stop=True at each 4-run end: sim-only flag;
                        # leaving the group open across the K/Q bank
                        # switches costs ~93ns per transition
                        nc.tensor.matmul(
                            bank[:, sl],
                            w_t[:, j * 128:(j + 1) * 128],
                            xT[:, j, i * 128:(i + 1) * 128],
                            start=(j == 0), stop=(jj == 3),
                            skip_group_check=True)

        def vnatural_group(c):
            # V in NATURAL [s(k), v] layout, one 128-col matmul per
            # (s-tile, e-chunk) with the xT slice as stationary:
            # out = x_stile @ Wv_chunk. Replaces the vT projection +
            # PE re-transpose; bias bv is folded into bo_eff instead
            # (see build_bo_bc), so no scalar add either.
            vp_ps = ps_mm.tile([128, 512], f32, tag="kqv", bufs=2,
                               name=f"vn{c}")
            for tt in range(4):
                i = c * 4 + tt
                for j in range(E_CHUNKS):
                    nc.tensor.matmul(
                        vp_ps[:, tt * 128:(tt + 1) * 128],
                        xT[:, j, i * 128:(i + 1) * 128],
                        wv_t[:, j * 128:(j + 1) * 128],
                        start=(j == 0), stop=(j == E_CHUNKS - 1))
            nc.vector.tensor_copy(
                v_big[:, c * 512:(c + 1) * 512], vp_ps[:])

        # ---- attention machinery (flat slot stream) ------------------
        hT = singles.tile([128, S], bf16)      # normalized H^T [v, q]

        outproj_q = []                         # out-proj closures
        pending_hr = []                        # (qq, t) H/rowsum slots
        chunk_state = {}                       # qq -> (h_ps, r_ps, p_ts, pq_ts, pquad_ts)

        def make_outproj(s0, half):
            def emit(pool=ps_mm, tag="kqv", queue=None):
                o_ps = pool.tile([128, 512], f32, tag=tag, bufs=2)
                nc.tensor.matmul(o_ps[:],
                                 hT[:, s0:s0 + 128],
                                 wo_t[:, half * 512:(half + 1) * 512],
                                 start=True, stop=True)
                o_sb = o_pool.tile([128, 512], f32, tag="o_sb")
                nc.vector.tensor_add(
                    o_sb[:], o_ps[:],
                    bo_bc[:, half * 512:(half + 1) * 512])
                (queue or nc.sync).dma_start(
                    out=out_ext[s0:s0 + 128,
                                half * 512:(half + 1) * 512],
                    in_=o_sb[:])
            return emit

        def emit_hr(qq, t):
            h_ps, r_ps, p_ts, pq_ts, pquad_ts = chunk_state[qq]
            # stop=True on every accumulation step: leaving the group
            # open across interleaved bank switches costs ~93 ns per
            # transition (PSUM pipeline flush); stop is sim-bookkeeping
            # only, accumulation continues via start=False.
            nc.tensor.matmul(h_ps[:], v_big[:, t * 128:(t + 1) * 128],
                             p_ts[t // 2][:, t % 2, :],
                             start=(t == 0), stop=True,
                             skip_group_check=True)
            if qq == N_QCHUNKS - 1 and t >= S_TILES - 4:
                # final four tiles of the final chunk: direct per-tile
                # rowsums so the normalize does not wait on the DVE
                # pre-sum chain at the very tail of the kernel
                nc.tensor.matmul(r_ps[:], ones_mat[:],
                                 p_ts[t // 2][:, t % 2, :],
                                 start=False, stop=True,
                                 skip_group_check=True)
            elif t % 2 == 1:
                # rowsum of a vector-presummed p-tile PAIR. (Pairs,
                # not quads: the mid-phase pacer is the SATURATED
                # vector engine, so the extra quad-level DVE add costs
                # more than the rowsum matmul it saves on the PE.)
                nc.tensor.matmul(r_ps[:], ones_mat[:], pq_ts[t // 2][:],
                                 start=(t == 1), stop=True,
                                 skip_group_check=True)
            if t == S_TILES - 1:
                finish_chunk(qq)

        def finish_chunk(qq):
            h_ps, r_ps, p_ts, pq_ts, pquad_ts = chunk_state[qq]
            qs = qq * Q_CHUNK
            # rowsum already broadcast across partitions; reciprocal
            # directly on the [128, Q_CHUNK] PSUM bank.
            r_bc = rs_pool.tile([128, Q_CHUNK], f32, tag="r_bc")
            nc.vector.reciprocal_approx_fast(r_bc[:], r_ps[:])
            # ONE [128,512] normalize mul (the DVE is the mid-phase
            # pacer; 1x ~690ns beats 4x ~500ns per-si muls)
            nc.vector.tensor_mul(hT[:, qs:qs + Q_CHUNK], h_ps[:], r_bc[:])
            for si in range(Q_CHUNK // 128):
                for half in range(2):
                    outproj_q.append(make_outproj(qs + si * 128, half))

        def emit_slot(qq, t):
            qs = qq * Q_CHUNK
            final_tail = (qq == N_QCHUNKS - 1 and t >= S_TILES - 4)
            s_ps = ps_s.tile([128, Q_CHUNK], f32, tag="s")
            nc.tensor.matmul(s_ps[:],
                             kT[:, t * 128:(t + 1) * 128],
                             qT[:, qs:qs + Q_CHUNK],
                             start=True, stop=True)
            if t % 2 == 0:
                pp = pt_pool.tile([128, 2, Q_CHUNK], bf16, tag="p",
                                  name=f"p{qq}_{t // 2}")
                chunk_state[qq][2].append(pp)
            p_t = chunk_state[qq][2][t // 2][:, t % 2, :]
            nc.scalar.activation(out=p_t, in_=s_ps[:], func=Exp,
                                 scale=SCALE)
            if t % 2 == 1 and not final_tail:
                # vector pre-sum of the p pair (bf16, ~350 ns)
                pq = pt_pool.tile([128, Q_CHUNK], bf16, tag="pq",
                                  name=f"pq{qq}_{t // 2}")
                chunk_state[qq][3].append(pq)
                # pair pre-sum stays on the DVE: gpsimd does this add
                # in 1155ns vs 415 (and contends for the shared SBUF
                # port) -- measured chunk cadence stretched ~0.5us
                nc.vector.tensor_add(
                    pq[:], chunk_state[qq][2][t // 2][:, 0, :],
                    chunk_state[qq][2][t // 2][:, 1, :])
            pending_hr.append((qq, t))
            if len(pending_hr) > LOOK:
                emit_hr(*pending_hr.pop(0))
            # out-projections of the previous chunk on the slots
            # whose H is even (no rowsum there); hold off until
            # slot 5 (its normalize lands ~slot 4) unless backlogged
            if outproj_q and ((t % 2 == 1 and t >= 5)
                              or len(outproj_q) > 2):
                outproj_q.pop(0)()

        def open_chunk(qq):
            h_ps = ps_h.tile([128, Q_CHUNK], f32, tag="h")
            r_ps = ps_r.tile([128, Q_CHUNK], f32, tag="r")
            chunk_state[qq] = (h_ps, r_ps, [], [], [])

        def finish_group(c):
            # chunk c's K/Q banks are fully accumulated: bias-add them
            # out (scalar), compute the group's natural-V tiles, then
            # emit the four chunk-0 attention slots this group unblocks
            nc.scalar.add(kT[:, c * 512:(c + 1) * 512],
                          kq_banks[("k", c)][:], bk_t[:])
            nc.scalar.add(qT[:, c * 512:(c + 1) * 512],
                          kq_banks[("q", c)][:], bq_t[:])
            vnatural_group(c)
            for t in range(4 * c, 4 * c + 4):
                emit_slot(0, t)

        # ---- stile pipeline: transposes + per-stile K/Q projections --
        # Stiles 0-3 project 0-behind (the head is DMA-paced, so the
        # projection matmuls fill the x-arrival gaps and keep the PE
        # p-state ramp alive); stiles 4+ project 1-behind their
        # transpose so the xT half-copies (DVE) are never waited on.
        open_chunk(0)
        for i in range(S_TILES):
            transpose_stile(i)
            pi = i if i < 4 else (i - 1 if i >= 5 else None)
            if pi is not None:
                proj_stile(pi)
                if pi % 4 == 3:
                    finish_group(pi // 4)
        proj_stile(S_TILES - 1)
        finish_group(N_QCHUNKS - 1)

        build_bo_bc()   # needs wo_t (landed ~35us); first use ~45us

        # ---- remaining attention chunks ------------------------------
        for qq in range(1, N_QCHUNKS):
            open_chunk(qq)
            for t in range(S_TILES):
                emit_slot(qq, t)

        while pending_hr:
            emit_hr(*pending_hr.pop(0))
        # drain: rotate o_ps across ps_mm + ps_s (5 PSUM banks) so the
        # matmuls pipeline ahead of their DVE consumers
        # drain: rotate PSUM rings AND alternate the two DMA queues
        # (gpsimd is idle at the tail) so the final writes drain in
        # parallel
        drain_pools = [(ps_mm, "kqv"), (ps_s, "s")]
        di = 0
        while outproj_q:
            pool, tag = drain_pools[di % len(drain_pools)]
            outproj_q.pop(0)(pool, tag,
                             nc.gpsimd if di % 2 else nc.sync)
            di += 1

    nc.compile()
    return nc


_NC = None


def kernel(**inputs):
    global _NC
    from concourse.bass_utils import run_bass_kernel_spmd

    if _NC is None:
        _NC = build_nc()

    x = np.asarray(inputs["embedding_matrix"], dtype=np.float32)
    shared = {k: np.ascontiguousarray(np.asarray(inputs[k], dtype=np.float32))
              for k in ("Wq", "bq", "Wk", "bk", "Wv", "bv", "Wo", "bo")}
    in_maps = [dict(shared, x=np.ascontiguousarray(x[c])) for c in range(N_CORES)]

    res = run_bass_kernel_spmd(_NC, in_maps, core_ids=list(range(N_CORES)))
    out = np.stack([res.results[c]["out"] for c in range(N_CORES)], axis=0)
    return out.astype(np.float32)
